# revision 1
# baseline (speedup 1.0000x reference)
"""Trainium2 Bass kernel for nn_Caps2dMatwo (capsule conv + dual routing).

Strategy (8 NeuronCores, no cross-core communication):
  - Shard: core k handles batch n=k//4, H-rows [32*(k%4), 32*(k%4)+32).
  - PE stage: the 3x3 conv and the capsule pose/appearance transforms are
    fused into 9 per-tap matmuls over a permuted 128-channel axis
    (i,c,q,j), block-diagonal per 32-row i-block (4 row-tiled matmuls per
    tap).  The PSUM output holds, per pixel: u_hat (t,c,pa,i,k), the
    iteration-1 routing input p1 = 0.5*sum_c u_hat, and raw j=3 conv
    taps used for the coordinate-addition fixup.
  - Routing (3 iterations, sigmoid coupling, psquash/matwo_squash) runs
    on DVE/ACT/GPSIMD with pixels on partitions and capsule dims on the
    free axis (segmented free-axis reduces).
  - Output row is PE-transposed to channel-major and DMA'd contiguously.
"""
import os
from contextlib import ExitStack

import numpy as np

# problem constants (hardcoded per spec)
N, T0, T1 = 2, 4, 8
H, W = 128, 128
PD, AD = 4, 4
Z = 32
NBLK = 360          # psum cols per i-block: 256 own + 64 usum + 32 craw + 8 csum
ROWS = 32           # output rows per core
P = 128

_CACHE = {}


# ----------------------------------------------------------------- host prep
def _build_weights(W_conv, W_pos, W_app, b_app):
    """W_eff for the fused conv+pose matmul.

    Returns:
      w_in    [128, 9, 360]  per-partition weights, partition = i*32+c*8+q*4+j
      bias_uh [4c, 8t, 4k]   appearance bias term  b_app * colsum(Mapp)
      bias_p1 [8t, 4k]       0.5 * sum_c bias_uh
    """
    Kc = np.asarray(W_conv, np.float64)[:, :, :, 0, :]          # [c,dy,dx,t1]
    Mpos = np.asarray(W_pos, np.float64).reshape(T0, T1, PD, PD).copy()
    Mpos = Mpos / np.sqrt(np.maximum((Mpos ** 2).sum(axis=2, keepdims=True), 1e-12))
    Mapp = np.asarray(W_app, np.float64).reshape(T0, T1, AD, AD)
    Sapp = Mapp.sum(axis=2)                                      # [c,t,k]

    W_eff = np.zeros((9, 128, 4, NBLK), np.float64)
    for tap in range(9):
        dy, dx = tap // 3, tap % 3
        for i in range(4):
            for c in range(4):
                for q in range(2):
                    for j in range(4):
                        row = i * 32 + c * 8 + q * 4 + j
                        for t in range(q, 8, 2):
                            kpos = Kc[c, dy, dx, t // 2]
                            kapp = Kc[c, dy, dx, 4 + t // 2]
                            base = t * 32 + c * 8
                            W_eff[tap, row, i, base:base + 4] = kpos * Mpos[c, t, j]
                            W_eff[tap, row, i, base + 4:base + 8] = kapp * Mapp[c, t, j]
                            ub = 256 + t * 8
                            W_eff[tap, row, i, ub:ub + 4] += 0.5 * kpos * Mpos[c, t, j]
                            W_eff[tap, row, i, ub + 4:ub + 8] += 0.5 * kapp * Mapp[c, t, j]
                            if j == 3:
                                W_eff[tap, row, i, 320 + t * 4 + c] = kpos
                                W_eff[tap, row, i, 352 + t] += 0.5 * kpos
    # [9, 128, 4, 360] -> [128, 9, 360] picking each partition's own block
    w_in = np.zeros((128, 9, NBLK), np.float32)
    for i in range(4):
        w_in[i * 32:(i + 1) * 32] = W_eff[:, i * 32:(i + 1) * 32, i, :].transpose(1, 0, 2)
    bias_uh = np.einsum('ct,ctk->ctk', np.asarray(b_app, np.float64), Sapp)
    bias_p1 = 0.5 * bias_uh.sum(axis=0)
    return w_in, bias_uh.astype(np.float32), bias_p1.astype(np.float32)


def _shard_x(x):
    """x [N,T0,Z,H,W] -> list of 8 arrays [128, 34*130] (permuted channels)."""
    xp = np.zeros((N, T0, Z, H + 2, W + 2), np.float32)
    xp[:, :, :, 1:H + 1, 1:W + 1] = np.asarray(x, np.float32)
    # z = q*16 + i*4 + j ; partition = i*32 + c*8 + q*4 + j
    xq = xp.reshape(N, T0, 2, 4, 4, H + 2, W + 2)                # n c q i j h w
    xperm = np.ascontiguousarray(xq.transpose(0, 3, 1, 2, 4, 5, 6)
                                 ).reshape(N, 128, H + 2, W + 2)
    shards = []
    for core in range(8):
        n, rb = core // 4, (core % 4) * 32
        shards.append(np.ascontiguousarray(
            xperm[n, :, rb:rb + 34, :]).reshape(128, 34 * 130))
    return shards


# ------------------------------------------------------------- bass module
def _build_module():
    import concourse.bass as bass
    import concourse.tile as tile
    import concourse.mybir as mybir
    from concourse import bacc

    f32 = mybir.dt.float32
    f16 = mybir.dt.float16
    AX = mybir.AxisListType.X
    OP = mybir.AluOpType
    AF = mybir.ActivationFunctionType

    nc = bacc.Bacc("TRN2", num_devices=8, debug=False)
    x_d = nc.dram_tensor("x_shard", [128, 34 * 130], f16, kind="ExternalInput").ap()
    w_d = nc.dram_tensor("w_eff", [128, 9, NBLK], f16, kind="ExternalInput").ap()
    buh_d = nc.dram_tensor("bias_uh", [128, 512], f16, kind="ExternalInput").ap()
    bp1_d = nc.dram_tensor("bias_p1", [128, 128], f16, kind="ExternalInput").ap()
    cxy_d = nc.dram_tensor("cxy", [128, ROWS * 2], f32, kind="ExternalInput").ap()
    out_d = nc.dram_tensor("out_shard", [256, ROWS * 128], f16,
                           kind="ExternalOutput").ap()

    GRP = 8  # rows emitted per software-pipeline wave (batches ACT table sets)

    with tile.TileContext(nc) as tc, ExitStack() as ctx:
        const = ctx.enter_context(tc.tile_pool(name="const", bufs=1))
        work = ctx.enter_context(tc.tile_pool(name="work", bufs=GRP + 1))
        small = ctx.enter_context(tc.tile_pool(name="small", bufs=GRP + 1))
        psum = ctx.enter_context(tc.tile_pool(name="psum", bufs=2, space="PSUM"))

        x_sb = const.tile([P, 34, 130], f16)
        nc.sync.dma_start(out=x_sb[:].rearrange("p a b -> p (a b)"), in_=x_d)
        w_sb = const.tile([P, 9, NBLK], f16)
        nc.sync.dma_start(out=w_sb, in_=w_d)
        buh = const.tile([P, 8, 4, 16], f16)      # (t, c, (i k)) app bias, i-expanded
        nc.sync.dma_start(out=buh[:].rearrange("p a b c -> p (a b c)"), in_=buh_d)
        bp1 = const.tile([P, 8, 16], f16)         # (t, (i k)) p1 app bias, i-expanded
        nc.sync.dma_start(out=bp1[:].rearrange("p a b -> p (a b)"), in_=bp1_d)
        cxy = const.tile([P, ROWS, 2], f32)       # per row: (w/128, h/128)
        nc.sync.dma_start(out=cxy[:].rearrange("p a b -> p (a b)"), in_=cxy_d)
        eps_t = const.tile([P, 1], f32)
        nc.vector.memset(eps_t, 1e-9)
        tbuf = const.tile([P, 2, ROWS, 128], f16)  # (ch-half, r, w) output staging

        st = {}  # per-row live tiles

        def s0_matmul(r):
            ups = psum.tile([P, 2048], f32, tag="ups")
            st[r] = {"ups": ups}
            for tap in range(9):
                dy, dx = tap // 3, tap % 3
                patch = x_sb[:, r + dy, dx:dx + 128]
                for i in range(4):
                    nc.tensor.matmul(
                        ups[:, i * 512:i * 512 + 360],
                        lhsT=patch[32 * i:32 * (i + 1), :],
                        rhs=w_sb[32 * i:32 * (i + 1), tap, :],
                        start=(tap == 0), stop=(tap == 8),
                        tile_position=(32 * i, 0))

        def s1_assemble(r):
            ups = st[r]["ups"]
            uh = work.tile([P, 2, 8, 4, 16], f16, tag="uh")    # (pa, t, c, ik)
            p1 = work.tile([P, 2, 8, 16], f16, tag="p")        # (pa, t, ik)
            for i in range(4):
                own = ups[:, i * 512:i * 512 + 256].rearrange(
                    "p (t c pa k) -> p pa t c k", t=8, c=4, pa=2)
                nc.scalar.copy(uh[:, :, :, :, i * 4:(i + 1) * 4], own)
                usum = ups[:, i * 512 + 256:i * 512 + 320].rearrange(
                    "p (t pa k) -> p pa t k", t=8, pa=2)
                nc.scalar.copy(p1[:, :, :, i * 4:(i + 1) * 4], usum)
            upsb = ups[:].rearrange("p (i n) -> p i n", i=4)
            craw = small.tile([P, 8, 4, 4], f16, tag="craw")   # (t, c, i)
            nc.scalar.copy(craw, upsb[:, :, 320:352].rearrange(
                "p i (t c) -> p t c i", t=8))
            csum = small.tile([P, 8, 4], f16, tag="csum")      # (t, i)
            nc.scalar.copy(csum, upsb[:, :, 352:360].transpose([0, 2, 1]))

            nc.gpsimd.tensor_add(uh[:, 1], uh[:, 1], buh[:])
            nc.gpsimd.tensor_add(p1[:, 1], p1[:, 1], bp1[:])
            # coordinate addition: +cx*craw into k=0, +cy*craw into k=1
            tmp1 = small.tile([P, 2, 128], f16, tag="tmp1")
            tmp2 = small.tile([P, 2, 32], f16, tag="tmp2")
            for k in (0, 1):
                sc = cxy[:, r, k:k + 1]
                nc.vector.tensor_scalar_mul(
                    tmp1[:, k], craw[:].rearrange("p t c i -> p (t c i)"), sc)
                nc.vector.tensor_scalar_mul(
                    tmp2[:, k], csum[:].rearrange("p t i -> p (t i)"), sc)
            uv = uh[:, 0].rearrange("p t c (i k) -> p (t c) i k", i=4)[:, :, :, 0:2]
            nc.vector.tensor_add(
                uv, uv, tmp1[:].rearrange("p k (tc i) -> p tc i k", i=4))
            pv = p1[:, 0].rearrange("p t (i k) -> p t i k", i=4)[:, :, :, 0:2]
            nc.vector.tensor_add(
                pv, pv, tmp2[:].rearrange("p k (t i) -> p t i k", i=4))
            st[r].update(uh=uh, p=p1)

        def squash(r, vpos, vapp, vjoint=None):
            p = st[r]["p"]
            md = small.tile([P, 2, 8], f32, tag="md")   # [mx | den]
            nc.vector.tensor_reduce(out=md[:, 0], in_=p[:, 0], axis=AX,
                                    op=OP.max, apply_absolute_value=True)
            sq = small.tile([P, 8, 16], f32, tag="sq")
            nc.scalar.square(sq, p[:, 1])
            s = small.tile([P, 8], f32, tag="s")
            nc.vector.tensor_reduce(out=s, in_=sq, axis=AX, op=OP.add)
            sq1 = small.tile([P, 8], f32, tag="sq1")
            nc.scalar.activation(sq1, s, AF.Sqrt, bias=eps_t[:, 0:1])
            nc.vector.scalar_tensor_tensor(out=md[:, 1], in0=s, scalar=1.0,
                                           in1=sq1, op0=OP.add, op1=OP.mult)
            facs = small.tile([P, 2, 8], f16, tag="facs")   # [rmx | gf]
            with nc.allow_low_precision("v factors consumed in fp16 anyway"):
                nc.vector.reciprocal(facs[:], md[:])
            nc.vector.tensor_mul(facs[:, 1], s, facs[:, 1])
            if vjoint is not None:
                nc.vector.tensor_mul(
                    vjoint, p[:], facs[:].unsqueeze(3).broadcast_to((P, 2, 8, 16)))
            else:
                nc.vector.tensor_mul(vpos, p[:, 0],
                                     facs[:, 0].unsqueeze(2).broadcast_to((P, 8, 16)))
                nc.vector.tensor_mul(vapp, p[:, 1],
                                     facs[:, 1].unsqueeze(2).broadcast_to((P, 8, 16)))

        def rout_badd(r, first):
            uh, v, b2 = st[r]["uh"], st[r]["v"], st[r].get("b2")
            uhm = uh[:].rearrange("p pa t c ik -> p (pa t) c ik")
            vm = (v[:].rearrange("p pa t ik -> p (pa t) ik")
                  .unsqueeze(2).broadcast_to((P, 16, 4, 16)))
            wp = work.tile([P, 16, 4, 16], f16, tag="wp")
            nc.vector.tensor_mul(wp, uhm, vm)
            wa = work.tile([P, 16, 4, 8], f16, tag="wa")
            nc.vector.tensor_add(wa, wp[:, :, :, 0:8], wp[:, :, :, 8:16])
            wb = work.tile([P, 16, 4, 4], f16, tag="wb")
            nc.vector.tensor_add(wb, wa[:, :, :, 0:4], wa[:, :, :, 4:8])
            wc = work.tile([P, 16, 4, 2], f16, tag="wc")
            nc.vector.tensor_add(wc, wb[:, :, :, 0:2], wb[:, :, :, 2:4])
            ab = small.tile([P, 2, 8, 4], f16, tag="ab")
            nc.vector.tensor_add(ab[:].rearrange("p pa t c -> p (pa t) c"),
                                 wc[:, :, :, 0], wc[:, :, :, 1])
            if first:
                b2 = work.tile([P, 2, 8, 4], f32, tag="b")
                st[r]["b2"] = b2
                nc.vector.tensor_mul(b2[:, 0], ab[:, 0], ab[:, 1])
                nc.vector.tensor_mul(b2[:, 1], ab[:, 0], ab[:, 1])
            else:
                rt = small.tile([P, 8, 4], f32, tag="rt")
                nc.vector.tensor_mul(rt, ab[:, 0], ab[:, 1])
                nc.vector.tensor_add(b2[:, 0], b2[:, 0], rt)
                nc.vector.tensor_add(b2[:, 1], b2[:, 1], rt)

        def sig_p(r, gp):
            uh, b2 = st[r]["uh"], st[r]["b2"]
            r2 = work.tile([P, 2, 8, 4], f16, tag="rt_sig")
            nc.scalar.activation(r2, b2, AF.Sigmoid)
            uhm = uh[:].rearrange("p pa t c ik -> p (pa t) c ik")
            rv = (r2[:].rearrange("p pa t c -> p (pa t) c")
                  .unsqueeze(3).broadcast_to((P, 16, 4, 16)))
            m = work.tile([P, 2, 8, 4, 16], f16, tag="m")
            p = work.tile([P, 2, 8, 16], f16, tag="p")
            eng = nc.gpsimd if gp else nc.vector
            eng.tensor_mul(
                m[:].rearrange("p pa t c ik -> p (pa t) c ik"), uhm, rv)
            ta = work.tile([P, 2, 8, 16], f16, tag="ta")
            eng.tensor_add(ta[:], m[:, :, :, 0, :], m[:, :, :, 1, :])
            tb = work.tile([P, 2, 8, 16], f16, tag="tb")
            eng.tensor_add(tb[:], m[:, :, :, 2, :], m[:, :, :, 3, :])
            eng.tensor_add(p[:], ta[:], tb[:])
            st[r]["p"] = p

        def s2_squash1_rout(r):
            v = work.tile([P, 2, 8, 16], f16, tag="v")
            st[r]["v"] = v
            squash(r, None, None, vjoint=v[:])
            rout_badd(r, first=True)

        def s3_sig_p2(r):
            sig_p(r, gp=False)

        def s4_squash2_rout(r):
            v = work.tile([P, 2, 8, 16], f16, tag="v")
            st[r]["v"] = v
            squash(r, None, None, vjoint=v[:])
            rout_badd(r, first=False)

        def s5_sig_p3(r):
            sig_p(r, gp=True)

        def s6_squash3_out(r):
            v3 = work.tile([P, 8, 2, 16], f16, tag="v3")
            squash(r, v3[:, :, 0, :], v3[:, :, 1, :])
            vflat = v3[:].rearrange("p t pa ik -> p (t pa ik)")
            for half in (0, 1):
                nc.sync.dma_start_transpose(
                    tbuf[:, half, r, :], vflat[:, half * 128:(half + 1) * 128])
            del st[r]

        stages = [s0_matmul, s1_assemble, s2_squash1_rout, s3_sig_p2,
                  s4_squash2_rout, s5_sig_p3, s6_squash3_out]
        for g in range(0, ROWS, GRP):
            rows = range(g, min(g + GRP, ROWS))
            for stage in stages:
                for r in rows:
                    stage(r)

        nc.sync.dma_start(out=out_d[0:128, :],
                          in_=tbuf[:, 0].rearrange("p a b -> p (a b)"))
        nc.sync.dma_start(out=out_d[128:256, :],
                          in_=tbuf[:, 1].rearrange("p a b -> p (a b)"))

    nc.compile()
    return nc


def _make_in_map(core, shards, w_in, bias_uh, bias_p1):
    """Per-core input dict. bias_uh [c,t,k] and bias_p1 [t,k] get i-expanded."""
    rb = (core % 4) * 32
    # (t, c, (i, k)) with i broadcast
    buh_in = np.broadcast_to(
        bias_uh.transpose(1, 0, 2)[:, :, None, :], (8, 4, 4, 4)).reshape(1, 512)
    buh_in = np.broadcast_to(buh_in, (128, 512)).copy()
    bp1_in = np.broadcast_to(
        bias_p1[:, None, :], (8, 4, 4)).reshape(1, 128)
    bp1_in = np.broadcast_to(bp1_in, (128, 128)).copy()
    cxy_in = np.zeros((128, ROWS, 2), np.float32)
    cxy_in[:, :, 0] = (np.arange(128, dtype=np.float32) / 128.0)[:, None]
    cxy_in[:, :, 1] = ((rb + np.arange(ROWS, dtype=np.float32)) / 128.0)[None, :]
    return {
        "x_shard": shards[core].astype(np.float16),
        "w_eff": w_in.astype(np.float16),
        "bias_uh": buh_in.astype(np.float16),
        "bias_p1": bp1_in.astype(np.float16),
        "cxy": cxy_in.reshape(128, ROWS * 2),
    }


def kernel(x, W_conv, W_pos, W_app, b_app):
    from concourse.bass_utils import run_bass_kernel_spmd

    if "nc" not in _CACHE:
        _CACHE["nc"] = _build_module()
    nc = _CACHE["nc"]

    w_in, bias_uh, bias_p1 = _build_weights(W_conv, W_pos, W_app, b_app)
    shards = _shard_x(x)
    in_maps = [_make_in_map(core, shards, w_in, bias_uh, bias_p1)
               for core in range(8)]

    trace = bool(int(os.environ.get("CAPS_TRACE", "0")))
    res = run_bass_kernel_spmd(nc, in_maps, core_ids=list(range(8)), trace=trace)
    _CACHE["last_result"] = res

    out = np.zeros((N, T1, Z, H, W), np.float32)
    for core in range(8):
        n, rb = core // 4, (core % 4) * 32
        o = res.results[core]["out_shard"].astype(np.float32).reshape(
            8, 32, ROWS, 128)
        out[n, :, :, rb:rb + 32, :] = o
    return out



# revision 13
# speedup vs baseline: 1.1838x; 1.1838x over previous
"""Trainium2 Bass kernel for nn_Caps2dMatwo (capsule conv + dual routing).

Strategy (8 NeuronCores, no cross-core communication):
  - Shard: core k handles batch n=k//4, H-rows [32*(k%4), 32*(k%4)+32).
  - PE stage: 3x3 conv + capsule pose/appearance transforms fused into 9
    per-tap matmuls over a permuted 128-channel axis, block-diagonal per
    32-row i-block.  PSUM column layout per i-block: (pa, t, c', k) with
    c'<4 the per-input-capsule u_hat and c'=4 the 0.5*sum_c partial (the
    iteration-1 routing p), then 40 cols of raw j=3 conv taps (t, c'')
    for the coordinate-addition fixup.
  - Routing (3 iterations) runs with R=8 rows batched per instruction:
    pixels on partitions, (row, pa, t, c, ik) on the free axis.  DVE does
    the heavy elementwise work, GPSIMD evacuates PSUM (folding the app
    bias into the evac add), ACT does only Sqrt/Sigmoid.
  - Agreements use raw p with factor scaling deferred past the reduce,
    so the ACT sqrt hides behind the big DVE ops.
  - Output is written w-major per core; the host transposes to
    channel-major when gathering.
"""
import os
from contextlib import ExitStack

import numpy as np

# problem constants (hardcoded per spec)
N, T0, T1 = 2, 4, 8
H, W = 128, 128
PD, AD = 4, 4
Z = 32
NBLK = 360          # psum cols per i-block: 320 (pa,t,c',k) + 40 (t,c'')
ROWS = 32           # output rows per core
P = 128
R = 8               # rows batched per routing group
NG = ROWS // R

_CACHE = {}


# ----------------------------------------------------------------- host prep
def _build_weights(W_conv, W_pos, W_app, b_app):
    """W_eff for the fused conv+pose matmul.

    Column layout within each i-block (360 cols):
      pa*160 + t*20 + c*4 + k   (c<4)  u_hat contribution
      pa*160 + t*20 + 16  + k          0.5 * sum_c u_hat  (iter-1 p)
      320 + t*5 + c             (c<4)  raw j=3 pos conv tap (coord fixup)
      320 + t*5 + 4                    0.5 * sum_c raw tap (p fixup)

    Returns:
      w_in [128, 9, 360]  per-partition weights, partition = i*32+c*8+q*4+j
      bapp [8, 5, 4]      app bias (t, c', k): c'<4 = b_app*colsum(Mapp),
                          c'=4 = 0.5*sum_c of that
    """
    Kc = np.asarray(W_conv, np.float64)[:, :, :, 0, :]          # [c,dy,dx,t1]
    Mpos = np.asarray(W_pos, np.float64).reshape(T0, T1, PD, PD).copy()
    Mpos = Mpos / np.sqrt(np.maximum((Mpos ** 2).sum(axis=2, keepdims=True), 1e-12))
    Mapp = np.asarray(W_app, np.float64).reshape(T0, T1, AD, AD)
    Sapp = Mapp.sum(axis=2)                                      # [c,t,k]

    W_eff = np.zeros((9, 128, 4, NBLK), np.float64)
    for tap in range(9):
        dy, dx = tap // 3, tap % 3
        for i in range(4):
            for c in range(4):
                for q in range(2):
                    for j in range(4):
                        row = i * 32 + c * 8 + q * 4 + j
                        for t in range(q, 8, 2):
                            kpos = Kc[c, dy, dx, t // 2]
                            kapp = Kc[c, dy, dx, 4 + t // 2]
                            for k in range(4):
                                W_eff[tap, row, i, t * 20 + c * 4 + k] = \
                                    kpos * Mpos[c, t, j, k]
                                W_eff[tap, row, i, 160 + t * 20 + c * 4 + k] = \
                                    kapp * Mapp[c, t, j, k]
                                W_eff[tap, row, i, t * 20 + 16 + k] += \
                                    0.5 * kpos * Mpos[c, t, j, k]
                                W_eff[tap, row, i, 160 + t * 20 + 16 + k] += \
                                    0.5 * kapp * Mapp[c, t, j, k]
                            if j == 3:
                                W_eff[tap, row, i, 320 + t * 5 + c] = kpos
                                W_eff[tap, row, i, 320 + t * 5 + 4] += 0.5 * kpos
    # [9, 128, 4, 360] -> [128, 9, 360] picking each partition's own block
    w_in = np.zeros((128, 9, NBLK), np.float32)
    for i in range(4):
        w_in[i * 32:(i + 1) * 32] = W_eff[:, i * 32:(i + 1) * 32, i, :].transpose(1, 0, 2)
    buh = np.einsum('ct,ctk->tck', np.asarray(b_app, np.float64), Sapp)  # [t,c,k]
    bapp = np.zeros((8, 5, 4), np.float64)
    bapp[:, :4] = buh
    bapp[:, 4] = 0.5 * buh.sum(axis=1)
    # i-expand: (t, c', ik) with value independent of i
    bapp16 = np.broadcast_to(bapp[:, :, None, :], (8, 5, 4, 4)).reshape(8, 5, 16)
    return w_in, np.ascontiguousarray(bapp16).astype(np.float32)


def _shard_x(x):
    """x [N,T0,Z,H,W] -> list of 8 arrays [128, 34*130] (permuted channels)."""
    xp = np.zeros((N, T0, Z, H + 2, W + 2), np.float32)
    xp[:, :, :, 1:H + 1, 1:W + 1] = np.asarray(x, np.float32)
    # z = q*16 + i*4 + j ; partition = i*32 + c*8 + q*4 + j
    xq = xp.reshape(N, T0, 2, 4, 4, H + 2, W + 2)                # n c q i j h w
    xperm = np.ascontiguousarray(xq.transpose(0, 3, 1, 2, 4, 5, 6)
                                 ).reshape(N, 128, H + 2, W + 2)
    shards = []
    for core in range(8):
        n, rb = core // 4, (core % 4) * 32
        shards.append(np.ascontiguousarray(
            xperm[n, :, rb:rb + 34, :]).reshape(128, 34 * 130))
    return shards


# ------------------------------------------------------------- bass module
def _build_module():
    import concourse.bass as bass
    import concourse.tile as tile
    import concourse.mybir as mybir
    from concourse import bacc

    f32 = mybir.dt.float32
    f16 = mybir.dt.float16
    AX = mybir.AxisListType.X
    OP = mybir.AluOpType
    AF = mybir.ActivationFunctionType

    nc = bacc.Bacc("TRN2", num_devices=8, debug=False)
    x_d = nc.dram_tensor("x_shard", [128, 34 * 130], f16, kind="ExternalInput").ap()
    w_d = nc.dram_tensor("w_eff", [128, 9, NBLK], f16, kind="ExternalInput").ap()
    bapp_d = nc.dram_tensor("bapp", [128, 8 * 5 * 16], f16, kind="ExternalInput").ap()
    cxy_d = nc.dram_tensor("cxy", [128, 1 + ROWS], f32, kind="ExternalInput").ap()
    out_d = nc.dram_tensor("out_shard", [128, ROWS * 256], f16,
                           kind="ExternalOutput").ap()

    with tile.TileContext(nc) as tc, ExitStack() as ctx:
        const = ctx.enter_context(tc.tile_pool(name="const", bufs=1))
        grp = ctx.enter_context(tc.tile_pool(name="grp", bufs=2))
        sm = ctx.enter_context(tc.tile_pool(name="sm", bufs=2))
        big = ctx.enter_context(tc.tile_pool(name="big", bufs=1))
        psum = ctx.enter_context(tc.tile_pool(name="psum", bufs=2, space="PSUM"))

        x_sb = const.tile([P, 34, 130], f16)
        nc.sync.dma_start(out=x_sb[:].rearrange("p a b -> p (a b)"), in_=x_d)
        w_sb = const.tile([P, 9, NBLK], f16)
        nc.sync.dma_start(out=w_sb, in_=w_d)
        bapp = const.tile([P, 8, 5, 16], f16)     # (t, c', ik) app bias, i-expanded
        nc.sync.dma_start(out=bapp[:].rearrange("p a b c -> p (a b c)"), in_=bapp_d)
        cxy = const.tile([P, 1 + ROWS], f32)      # [cx | cy per row]
        nc.sync.dma_start(out=cxy, in_=cxy_d)
        eps_t = const.tile([P, 1], f32)
        nc.vector.memset(eps_t, 1e-9)

        st = {}  # per-group live tiles

        def mm_evac(g):
            """PE matmuls + ACT PSUM evacuation for group g's rows."""
            up1 = grp.tile([P, R, 2, 8, 5, 16], f16, tag="up1")
            crw = grp.tile([P, R, 8, 5, 4], f16, tag="crw")
            st[g] = {"up1": up1, "crw": crw}
            for j in range(R):
                r = g * R + j
                ps = psum.tile([P, 4, 512], f32, tag="ps")
                for tap in range(9):
                    dy, dx = tap // 3, tap % 3
                    patch = x_sb[:, r + dy, dx:dx + 128]
                    for i in range(4):
                        nc.tensor.matmul(
                            ps[:, i, 0:NBLK],
                            lhsT=patch[32 * i:32 * (i + 1), :],
                            rhs=w_sb[32 * i:32 * (i + 1), tap, :],
                            start=(tap == 0), stop=(tap == 8),
                            tile_position=(32 * i, 0))
                # evac: (i, (t c'), k) views — 3 free dims on both sides
                src = ps[:, :, 0:320].rearrange("p i (pa tc k) -> p pa i tc k",
                                                pa=2, tc=40)
                dst = up1[:, j].rearrange("p pa t c (i k) -> p pa i (t c) k", i=4)
                nc.scalar.copy(dst[:, 0], src[:, 0])
                nc.scalar.copy(dst[:, 1], src[:, 1])
                nc.scalar.copy(
                    crw[:, j],
                    ps[:, :, 320:360].rearrange("p i (t c) -> p t c i", t=8))

        def prefix(g):
            """GPSIMD: app bias add + coordinate scaling (cx, cy per row)."""
            up1, crw = st[g]["up1"], st[g]["crw"]
            uapp = up1[:, :, 1].rearrange("p r t c ik -> p r t (c ik)")
            nc.gpsimd.tensor_add(
                uapp, uapp,
                bapp[:].rearrange("p t c ik -> p t (c ik)")
                .unsqueeze(1).broadcast_to((P, R, 8, 80)))
            tmp = sm.tile([P, 2, R, 8, 5, 4], f16, tag="tmp")
            nc.gpsimd.tensor_scalar_mul(
                tmp[:, 0].rearrange("p r t c i -> p (r t c i)"),
                crw[:].rearrange("p r t c i -> p (r t c i)"), cxy[:, 0:1])
            cyg = (cxy[:, 1 + g * R:1 + (g + 1) * R]
                   .rearrange("p r -> p r ()")
                   .broadcast_to((P, R, 160)))
            nc.gpsimd.tensor_mul(
                tmp[:, 1].rearrange("p r t c i -> p r (t c i)"),
                crw[:].rearrange("p r t c i -> p r (t c i)"), cyg)
            st[g]["tmp"] = tmp

        def fixup(g):
            """DVE: add coordinate terms into u_hat pos (k=0,1) and p1.

            The c'=4 (p1) slot rides along: tmp's c''=4 holds the scaled
            0.5*sum_c raw tap, exactly the p1 coordinate term.
            """
            up1, tmp = st[g]["up1"], st[g]["tmp"]
            uh_ik = up1[:, :, 0].rearrange("p r t c (i k) -> p r (t c) i k", i=4)
            for k in (0, 1):
                nc.vector.tensor_add(
                    uh_ik[:, :, :, :, k], uh_ik[:, :, :, :, k],
                    tmp[:, k].rearrange("p r t c i -> p r (t c) i"))

        def squash_a(g, ppos, papp, it):
            """DVE reduces + ACT sqrt (issued early; f finished in squash_b)."""
            md = sm.tile([P, R, 2, 8], f32, tag=f"md{it}")
            nc.vector.tensor_reduce(out=md[:, :, 0], in_=ppos, axis=AX,
                                    op=OP.max, apply_absolute_value=True)
            sq = sm.tile([P, R, 8, 16], f32, tag="sq")
            nc.vector.tensor_mul(sq, papp, papp)
            s = sm.tile([P, R, 8], f32, tag=f"s{it}")
            nc.vector.tensor_reduce(out=s, in_=sq, axis=AX, op=OP.add)
            sq1 = sm.tile([P, R, 8], f32, tag=f"sq1{it}")
            nc.scalar.activation(sq1, s, AF.Sqrt, bias=eps_t[:, 0:1])
            return md, s, sq1

        def squash_b(g, md, s, sq1):
            """DVE: den = (1+s)*sqrt, f = [1/mx | s/den]."""
            nc.vector.scalar_tensor_tensor(out=md[:, :, 1], in0=s, scalar=1.0,
                                           in1=sq1, op0=OP.add, op1=OP.mult)
            f = sm.tile([P, R, 2, 8], f16, tag="f")
            with nc.allow_low_precision("f consumed in fp16 muls"):
                nc.vector.reciprocal(f[:], md[:])
            nc.vector.tensor_mul(f[:, :, 1], s, f[:, :, 1])
            return f

        def wp_ab(g, pv):
            """DVE: wp = uh*p (bcast over c), ab = sum_ik wp."""
            up1 = st[g]["up1"]
            uh = (up1[:, :, :, :, 0:4, :]
                  .rearrange("p r pa t c ik -> p (r pa t) c ik"))
            pm = (pv.rearrange("p r pa t ik -> p (r pa t) () ik")
                  .broadcast_to((P, R * 16, 4, 16)))
            wp = big.tile([P, R * 16, 4, 16], f16, tag="wp")
            nc.vector.tensor_mul(wp, uh, pm)
            ab = sm.tile([P, R * 16, 4], f32, tag="ab")
            nc.vector.tensor_reduce(out=ab, in_=wp, axis=AX, op=OP.add)
            return ab

        def badd(g, ab, f, first):
            """DVE: rt = (f_pos*ab_pos)*(f_app*ab_app); b (+)= rt."""
            t1 = sm.tile([P, R, 2, 8, 4], f16, tag="t1")
            nc.vector.tensor_mul(
                t1[:].rearrange("p r pa t c -> p (r pa) t c"),
                ab[:].rearrange("p (rpa t) c -> p rpa t c", t=8),
                f[:].rearrange("p r pa t -> p (r pa) t ()")
                .broadcast_to((P, R * 2, 8, 4)))
            if first:
                b = sm.tile([P, R, 8, 4], f32, tag="b")
                st[g]["b"] = b
                nc.vector.tensor_mul(b, t1[:, :, 0], t1[:, :, 1])
            else:
                b = st[g]["b"]
                rt = sm.tile([P, R, 8, 4], f32, tag="rt")
                nc.vector.tensor_mul(rt, t1[:, :, 0], t1[:, :, 1])
                nc.vector.tensor_add(b, b, rt)

        def sig_p(g, last):
            """ACT sigmoid (ik-expanded) + DVE: p = sum_c uh*r.

            Returns p as [P,R,2,8,16] (pa-major), or for `last` as
            [P,R,8,2,16] (t-major, the output channel layout).
            """
            up1, b = st[g]["up1"], st[g]["b"]
            r2x = sm.tile([P, R, 8, 4, 16], f16, tag="r2x")
            nc.scalar.activation(
                r2x[:].rearrange("p r t c ik -> p r (t c) ik"),
                b[:].rearrange("p r t c -> p r (t c) ()")
                .broadcast_to((P, R, 32, 16)), AF.Sigmoid)
            m = big.tile([P, R, 2, 8, 4, 16], f16, tag="m")
            for pa in (0, 1):
                nc.vector.tensor_mul(
                    m[:, :, pa].rearrange("p r t c ik -> p r t (c ik)"),
                    up1[:, :, pa, :, 0:4, :].rearrange(
                        "p r t c ik -> p r t (c ik)"),
                    r2x[:].rearrange("p r t c ik -> p r t (c ik)"))
            mm = m[:].rearrange("p r pa t c ik -> p (r pa) t c ik")
            ta = sm.tile([P, R, 2, 8, 16], f16, tag="ta")
            tb = sm.tile([P, R, 2, 8, 16], f16, tag="tb")
            tam = ta[:].rearrange("p r pa t ik -> p (r pa) t ik")
            tbm = tb[:].rearrange("p r pa t ik -> p (r pa) t ik")
            nc.vector.tensor_add(tam, mm[:, :, :, 0], mm[:, :, :, 1])
            nc.vector.tensor_add(tbm, mm[:, :, :, 2], mm[:, :, :, 3])
            if not last:
                p = sm.tile([P, R, 2, 8, 16], f16, tag="p")
                nc.vector.tensor_add(
                    p[:].rearrange("p r pa t ik -> p (r pa) t ik"), tam, tbm)
            else:
                p = grp.tile([P, R, 8, 2, 16], f16, tag="p3")
                for pa in (0, 1):
                    nc.vector.tensor_add(p[:, :, :, pa], ta[:, :, pa],
                                         tb[:, :, pa])
            return p

        def routing(g):
            up1 = st[g]["up1"]
            fixup(g)
            p1 = up1[:, :, :, :, 4, :]                  # [P,R,2,8,16] view
            md1, s1, sq11 = squash_a(g, p1[:, :, 0], p1[:, :, 1], 1)
            ab1 = wp_ab(g, p1)                          # sqrt runs under wp/ab
            f1 = squash_b(g, md1, s1, sq11)
            badd(g, ab1, f1, first=True)
            p2 = sig_p(g, last=False)
            md2, s2, sq12 = squash_a(g, p2[:, :, 0], p2[:, :, 1], 2)
            ab2 = wp_ab(g, p2)
            f2 = squash_b(g, md2, s2, sq12)
            badd(g, ab2, f2, first=False)
            p3 = sig_p(g, last=True)                    # [P,R,8,2,16] t-major
            md3, s3, sq13 = squash_a(g, p3[:, :, :, 0], p3[:, :, :, 1], 3)
            f3 = squash_b(g, md3, s3, sq13)
            v3 = grp.tile([P, R, 8, 2, 16], f16, tag="v3")
            for pa in (0, 1):
                nc.vector.tensor_mul(
                    v3[:, :, :, pa], p3[:, :, :, pa],
                    f3[:, :, pa].unsqueeze(3).broadcast_to((P, R, 8, 16)))
            nc.sync.dma_start(
                out=out_d[:, g * R * 256:(g + 1) * R * 256],
                in_=v3[:].rearrange("p a b c d -> p (a b c d)"))
            del st[g]

        for g in range(NG):
            mm_evac(g)
            prefix(g)
            routing(g)

    nc.compile()
    return nc


def _make_in_map(core, shards, w_in, bapp):
    rb = (core % 4) * 32
    cxy_in = np.zeros((128, 1 + ROWS), np.float32)
    cxy_in[:, 0] = np.arange(128, dtype=np.float32) / 128.0
    cxy_in[:, 1:] = ((rb + np.arange(ROWS, dtype=np.float32)) / 128.0)[None, :]
    bapp_in = np.broadcast_to(bapp.reshape(1, 640), (128, 640))
    return {
        "x_shard": shards[core].astype(np.float16),
        "w_eff": w_in.astype(np.float16),
        "bapp": np.ascontiguousarray(bapp_in).astype(np.float16),
        "cxy": cxy_in,
    }


def kernel(x, W_conv, W_pos, W_app, b_app):
    from concourse.bass_utils import run_bass_kernel_spmd

    if "nc" not in _CACHE:
        _CACHE["nc"] = _build_module()
    nc = _CACHE["nc"]

    w_in, bapp = _build_weights(W_conv, W_pos, W_app, b_app)
    shards = _shard_x(x)
    in_maps = [_make_in_map(core, shards, w_in, bapp) for core in range(8)]

    trace = bool(int(os.environ.get("CAPS_TRACE", "0")))
    res = run_bass_kernel_spmd(nc, in_maps, core_ids=list(range(8)), trace=trace)
    _CACHE["last_result"] = res

    out = np.zeros((N, T1, Z, H, W), np.float32)
    for core in range(8):
        n, rb = core // 4, (core % 4) * 32
        o = res.results[core]["out_shard"].astype(np.float32).reshape(
            128, ROWS, 8, 2, 16)
        # [w, r, t, pa, ik] -> [t, pa*16+ik, r, w]
        out[n, :, :, rb:rb + 32, :] = o.transpose(2, 3, 4, 1, 0).reshape(
            8, 32, ROWS, 128)
    return out


# revision 22
# speedup vs baseline: 1.6069x; 1.3574x over previous
"""Trainium2 Bass kernel for nn_Caps2dMatwo (capsule conv + dual routing).

Strategy (8 NeuronCores, no cross-core communication):
  - Shard: core k handles batch n=k//4, H-rows [32*(k%4), 32*(k%4)+32).
  - PE stage: 3x3 conv + capsule pose/appearance transforms fused into 9
    per-tap matmuls over a permuted 128-channel axis, block-diagonal per
    32-row i-block.  PSUM column layout per i-block: (pa, t, c', k) with
    c'<4 the per-input-capsule u_hat and c'=4 the 0.5*sum_c partial (the
    iteration-1 routing p), then 40 cols of raw j=3 conv taps (t, c'')
    for the coordinate-addition fixup.
  - Routing (3 iterations) runs with R=8 rows batched per instruction:
    pixels on partitions, (row, pa, t, c, ik) on the free axis.  DVE does
    the heavy elementwise work, GPSIMD evacuates PSUM (folding the app
    bias into the evac add), ACT does only Sqrt/Sigmoid.
  - Agreements use raw p with factor scaling deferred past the reduce,
    so the ACT sqrt hides behind the big DVE ops.
  - Output is written w-major per core; the host transposes to
    channel-major when gathering.
"""
import os
from contextlib import ExitStack

import numpy as np

# problem constants (hardcoded per spec)
N, T0, T1 = 2, 4, 8
H, W = 128, 128
PD, AD = 4, 4
Z = 32
NBLK = 360          # psum cols per i-block: 320 (pa,t,c',k) + 40 (t,c'')
ROWS = 32           # output rows per core
P = 128
R = 8               # rows batched per routing group
NG = ROWS // R

_CACHE = {}


# ----------------------------------------------------------------- host prep
def _build_weights(W_conv, W_pos, W_app, b_app):
    """W_eff for the fused conv+pose matmul.

    Column layout within each i-block (360 cols):
      pa*160 + t*20 + c*4 + k   (c<4)  u_hat contribution
      pa*160 + t*20 + 16  + k          0.5 * sum_c u_hat  (iter-1 p)
      320 + t*5 + c             (c<4)  raw j=3 pos conv tap (coord fixup)
      320 + t*5 + 4                    0.5 * sum_c raw tap (p fixup)

    Returns:
      w_in [128, 9, 360]  per-partition weights, partition = i*32+c*8+q*4+j
      bapp [8, 5, 4]      app bias (t, c', k): c'<4 = b_app*colsum(Mapp),
                          c'=4 = 0.5*sum_c of that
    """
    Kc = np.asarray(W_conv, np.float64)[:, :, :, 0, :]          # [c,dy,dx,t1]
    Mpos = np.asarray(W_pos, np.float64).reshape(T0, T1, PD, PD).copy()
    Mpos = Mpos / np.sqrt(np.maximum((Mpos ** 2).sum(axis=2, keepdims=True), 1e-12))
    Mapp = np.asarray(W_app, np.float64).reshape(T0, T1, AD, AD)
    Sapp = Mapp.sum(axis=2)                                      # [c,t,k]

    W_eff = np.zeros((9, 128, 4, NBLK), np.float64)
    for tap in range(9):
        dy, dx = tap // 3, tap % 3
        for i in range(4):
            for c in range(4):
                for q in range(2):
                    for j in range(4):
                        row = i * 32 + c * 8 + q * 4 + j
                        for t in range(q, 8, 2):
                            kpos = Kc[c, dy, dx, t // 2]
                            kapp = Kc[c, dy, dx, 4 + t // 2]
                            for k in range(4):
                                W_eff[tap, row, i, t * 20 + c * 4 + k] = \
                                    kpos * Mpos[c, t, j, k]
                                W_eff[tap, row, i, 160 + t * 20 + c * 4 + k] = \
                                    kapp * Mapp[c, t, j, k]
                                W_eff[tap, row, i, t * 20 + 16 + k] += \
                                    0.5 * kpos * Mpos[c, t, j, k]
                                W_eff[tap, row, i, 160 + t * 20 + 16 + k] += \
                                    0.5 * kapp * Mapp[c, t, j, k]
                            if j == 3:
                                W_eff[tap, row, i, 320 + t * 5 + c] = kpos
                                W_eff[tap, row, i, 320 + t * 5 + 4] += 0.5 * kpos
    # [9, 128, 4, 360] -> [128, 9, 360] picking each partition's own block
    w_in = np.zeros((128, 9, NBLK), np.float32)
    for i in range(4):
        w_in[i * 32:(i + 1) * 32] = W_eff[:, i * 32:(i + 1) * 32, i, :].transpose(1, 0, 2)
    # bias "ones tap" weights: app-half columns get bias/32 (32 partitions
    # of ones contract to exactly bias; /32 is exact in fp16)
    buh = np.einsum('ct,ctk->tck', np.asarray(b_app, np.float64), Sapp)  # [t,c,k]
    wb = np.zeros((NBLK,), np.float64)
    for t in range(8):
        for c in range(4):
            wb[160 + t * 20 + c * 4:160 + t * 20 + c * 4 + 4] = buh[t, c]
        wb[160 + t * 20 + 16:160 + t * 20 + 20] = 0.5 * buh[t].sum(axis=0)
    wb_in = np.broadcast_to((wb / 32.0)[None, :], (128, NBLK))
    return w_in, np.ascontiguousarray(wb_in).astype(np.float32)


def _shard_x(x):
    """x [N,T0,Z,H,W] -> list of 8 arrays [128, 34*130] (permuted channels)."""
    xp = np.zeros((N, T0, Z, H + 2, W + 2), np.float32)
    xp[:, :, :, 1:H + 1, 1:W + 1] = np.asarray(x, np.float32)
    # z = q*16 + i*4 + j ; partition = i*32 + c*8 + q*4 + j
    xq = xp.reshape(N, T0, 2, 4, 4, H + 2, W + 2)                # n c q i j h w
    xperm = np.ascontiguousarray(xq.transpose(0, 3, 1, 2, 4, 5, 6)
                                 ).reshape(N, 128, H + 2, W + 2)
    shards = []
    for core in range(8):
        n, rb = core // 4, (core % 4) * 32
        shards.append(np.ascontiguousarray(
            xperm[n, :, rb:rb + 34, :]).reshape(128, 34 * 130))
    return shards


# ------------------------------------------------------------- bass module
def _build_module():
    import concourse.bass as bass
    import concourse.tile as tile
    import concourse.mybir as mybir
    from concourse import bacc

    f32 = mybir.dt.float32
    f16 = mybir.dt.float16
    AX = mybir.AxisListType.X
    OP = mybir.AluOpType
    AF = mybir.ActivationFunctionType

    nc = bacc.Bacc("TRN2", num_devices=8, debug=False)
    x_d = nc.dram_tensor("x_shard", [128, 34 * 130], f16, kind="ExternalInput").ap()
    w_d = nc.dram_tensor("w_eff", [128, 9, NBLK], f16, kind="ExternalInput").ap()
    bapp_d = nc.dram_tensor("bapp", [128, NBLK], f16, kind="ExternalInput").ap()
    cxy_d = nc.dram_tensor("cxy", [128, 1 + ROWS], f32, kind="ExternalInput").ap()
    out_d = nc.dram_tensor("out_shard", [128, ROWS * 256], f16,
                           kind="ExternalOutput").ap()

    with tile.TileContext(nc) as tc, ExitStack() as ctx:
        const = ctx.enter_context(tc.tile_pool(name="const", bufs=1))
        grp = ctx.enter_context(tc.tile_pool(name="grp", bufs=2))
        sm = ctx.enter_context(tc.tile_pool(name="sm", bufs=2))
        big = ctx.enter_context(tc.tile_pool(name="big", bufs=1))
        psum = ctx.enter_context(tc.tile_pool(name="psum", bufs=2, space="PSUM"))

        x_sb = const.tile([P, 34, 130], f16)
        nc.sync.dma_start(out=x_sb[:].rearrange("p a b -> p (a b)"), in_=x_d)
        w_sb = const.tile([P, 9, NBLK], f16)
        nc.sync.dma_start(out=w_sb, in_=w_d)
        wb_sb = const.tile([P, NBLK], f16)        # bias/32 "ones tap" weights
        nc.sync.dma_start(out=wb_sb, in_=bapp_d)
        cxy = const.tile([P, 1 + ROWS], f32)      # [cx | cy per row]
        nc.sync.dma_start(out=cxy, in_=cxy_d)
        eps_t = const.tile([P, 1], f32)
        nc.vector.memset(eps_t, 1e-9)
        ones_sb = const.tile([P, 128], f16)
        nc.vector.memset(ones_sb, 1.0)

        st = {}  # per-group live tiles

        def mm_evac(g):
            """PE matmuls + ACT PSUM evacuation for group g's rows."""
            up1 = grp.tile([P, R, 2, 8, 5, 16], f16, tag="up1")
            crw = grp.tile([P, R, 8, 5, 4], f16, tag="crw")
            st[g] = {"up1": up1, "crw": crw}
            for j in range(R):
                r = g * R + j
                ps = psum.tile([P, 4, 512], f32, tag="ps")
                for tap in range(9):
                    dy, dx = tap // 3, tap % 3
                    patch = x_sb[:, r + dy, dx:dx + 128]
                    for i in range(4):
                        nc.tensor.matmul(
                            ps[:, i, 0:NBLK],
                            lhsT=patch[32 * i:32 * (i + 1), :],
                            rhs=w_sb[32 * i:32 * (i + 1), tap, :],
                            start=(tap == 0), stop=False,
                            tile_position=(32 * i, 0))
                for i in range(4):      # bias tap: sum_p 1*(bias/32) = bias
                    nc.tensor.matmul(
                        ps[:, i, 0:NBLK],
                        lhsT=ones_sb[32 * i:32 * (i + 1), :],
                        rhs=wb_sb[32 * i:32 * (i + 1), :],
                        start=False, stop=True,
                        tile_position=(32 * i, 0))
                # evac: (i, (t c'), k) views — 3 free dims on both sides
                src = ps[:, :, 0:320].rearrange("p i (pa tc k) -> p pa i tc k",
                                                pa=2, tc=40)
                dst = up1[:, j].rearrange("p pa t c (i k) -> p pa i (t c) k", i=4)
                nc.scalar.copy(dst[:, 0], src[:, 0])
                nc.scalar.copy(dst[:, 1], src[:, 1])
                nc.scalar.copy(
                    crw[:, j],
                    ps[:, :, 320:360].rearrange("p i (t c) -> p t c i", t=8))

        def prefix(g):
            """GPSIMD: coordinate-addition fixup into u_hat pos (k=0,1).

            The c'=4 (p1) slot rides along: tmp's c''=4 holds the scaled
            0.5*sum_c raw tap, exactly the p1 coordinate term.  All TT ops
            (TensorScalarPtr is ~100x slower on GPSIMD).
            """
            up1, crw = st[g]["up1"], st[g]["crw"]
            tmp = sm.tile([P, 2, R, 8, 5, 4], f16, tag="tmp")
            crf = crw[:].rearrange("p r t c i -> p (r t c i)")
            nc.gpsimd.tensor_mul(
                tmp[:, 0].rearrange("p r t c i -> p (r t c i)"),
                crf, cxy[:, 0:1].broadcast_to((P, R * 160)))
            cyg = (cxy[:, 1 + g * R:1 + (g + 1) * R]
                   .rearrange("p r -> p r ()")
                   .broadcast_to((P, R, 160)))
            nc.gpsimd.tensor_mul(
                tmp[:, 1].rearrange("p r t c i -> p r (t c i)"),
                crw[:].rearrange("p r t c i -> p r (t c i)"), cyg)
            uh_ik = up1[:, :, 0].rearrange("p r t c (i k) -> p r (t c) i k", i=4)
            for k in (0, 1):
                nc.gpsimd.tensor_add(
                    uh_ik[:, :, :, :, k], uh_ik[:, :, :, :, k],
                    tmp[:, k].rearrange("p r t c i -> p r (t c) i"))

        def squash_a(g, ppos, papp, it):
            """DVE reduces + ACT sqrt (issued early; f finished in squash_b)."""
            md = sm.tile([P, R, 2, 8], f32, tag=f"md{it}")
            nc.vector.tensor_reduce(out=md[:, :, 0], in_=ppos, axis=AX,
                                    op=OP.max, apply_absolute_value=True)
            sq = big.tile([P, R, 8, 16], f32, tag="sq")
            nc.vector.tensor_mul(sq, papp, papp)
            s = sm.tile([P, R, 8], f32, tag=f"s{it}")
            nc.vector.tensor_reduce(out=s, in_=sq, axis=AX, op=OP.add)
            sq1 = sm.tile([P, R, 8], f32, tag=f"sq1{it}")
            nc.scalar.activation(sq1, s, AF.Sqrt, bias=eps_t[:, 0:1])
            return md, s, sq1

        def squash_b(g, md, s, sq1):
            """DVE: den = (1+s)*sqrt, f = [1/mx | s/den]."""
            nc.vector.scalar_tensor_tensor(out=md[:, :, 1], in0=s, scalar=1.0,
                                           in1=sq1, op0=OP.add, op1=OP.mult)
            f = sm.tile([P, R, 2, 8], f16, tag="f")
            with nc.allow_low_precision("f consumed in fp16 muls"):
                nc.vector.reciprocal(f[:], md[:])
            nc.vector.tensor_mul(f[:, :, 1], s, f[:, :, 1])
            return f

        def wp_ab(g, pv):
            """DVE: wp = uh*p (bcast over c), ab = sum_ik wp (tree adds)."""
            up1 = st[g]["up1"]
            uh = (up1[:, :, :, :, 0:4, :]
                  .rearrange("p r pa t c ik -> p (r pa t) c ik"))
            pm = (pv.rearrange("p r pa t ik -> p (r pa t) () ik")
                  .broadcast_to((P, R * 16, 4, 16)))
            wp = big.tile([P, R * 16, 4, 16], f16, tag="wp")
            nc.vector.tensor_mul(wp, uh, pm)
            wa = big.tile([P, R * 16, 4, 8], f16, tag="wa")
            nc.vector.tensor_add(wa, wp[:, :, :, 0:8], wp[:, :, :, 8:16])
            wb = big.tile([P, R * 16, 4, 4], f16, tag="wb")
            nc.vector.tensor_add(wb, wa[:, :, :, 0:4], wa[:, :, :, 4:8])
            wc = big.tile([P, R * 16, 4, 2], f16, tag="wc")
            nc.vector.tensor_add(wc, wb[:, :, :, 0:2], wb[:, :, :, 2:4])
            ab = sm.tile([P, R * 16, 4], f32, tag="ab")
            nc.vector.tensor_add(ab, wc[:, :, :, 0], wc[:, :, :, 1])
            return ab

        def badd(g, ab, f, first):
            """DVE: rt = (f_pos*ab_pos)*(f_app*ab_app); b (+)= rt."""
            t1 = sm.tile([P, R, 2, 8, 4], f16, tag="t1")
            nc.vector.tensor_mul(
                t1[:].rearrange("p r pa t c -> p (r pa) t c"),
                ab[:].rearrange("p (rpa t) c -> p rpa t c", t=8),
                f[:].rearrange("p r pa t -> p (r pa) t ()")
                .broadcast_to((P, R * 2, 8, 4)))
            if first:
                b = sm.tile([P, R, 8, 4], f32, tag="b")
                st[g]["b"] = b
                nc.vector.tensor_mul(b, t1[:, :, 0], t1[:, :, 1])
            else:
                b = st[g]["b"]
                rt = sm.tile([P, R, 8, 4], f32, tag="rt")
                nc.vector.tensor_mul(rt, t1[:, :, 0], t1[:, :, 1])
                nc.vector.tensor_add(b, b, rt)

        def sig_p(g, last):
            """ACT sigmoid (ik-expanded) + DVE: p = sum_c uh*r.

            Returns p as [P,R,2,8,16] (pa-major), or for `last` as
            [P,R,8,2,16] (t-major, the output channel layout).
            """
            up1, b = st[g]["up1"], st[g]["b"]
            r2x = big.tile([P, R, 8, 4, 16], f16, tag="r2x")
            nc.scalar.activation(
                r2x[:].rearrange("p r t c ik -> p r (t c) ik"),
                b[:].rearrange("p r t c -> p r (t c) ()")
                .broadcast_to((P, R, 32, 16)), AF.Sigmoid)
            m = big.tile([P, R, 2, 8, 4, 16], f16, tag="m")
            for pa in (0, 1):
                nc.vector.tensor_mul(
                    m[:, :, pa].rearrange("p r t c ik -> p r t (c ik)"),
                    up1[:, :, pa, :, 0:4, :].rearrange(
                        "p r t c ik -> p r t (c ik)"),
                    r2x[:].rearrange("p r t c ik -> p r t (c ik)"))
            mm = m[:].rearrange("p r pa t c ik -> p (r pa) t c ik")
            ta = big.tile([P, R, 2, 8, 16], f16, tag="ta")
            tb = big.tile([P, R, 2, 8, 16], f16, tag="tb")
            tam = ta[:].rearrange("p r pa t ik -> p (r pa) t ik")
            tbm = tb[:].rearrange("p r pa t ik -> p (r pa) t ik")
            nc.vector.tensor_add(tam, mm[:, :, :, 0], mm[:, :, :, 1])
            nc.vector.tensor_add(tbm, mm[:, :, :, 2], mm[:, :, :, 3])
            if not last:
                p = sm.tile([P, R, 2, 8, 16], f16, tag="p")
                nc.vector.tensor_add(
                    p[:].rearrange("p r pa t ik -> p (r pa) t ik"), tam, tbm)
            else:
                p = grp.tile([P, R, 8, 2, 16], f16, tag="p3")
                for pa in (0, 1):
                    nc.vector.tensor_add(p[:, :, :, pa], ta[:, :, pa],
                                         tb[:, :, pa])
            return p

        def routing(g):
            up1 = st[g]["up1"]
            p1 = up1[:, :, :, :, 4, :]                  # [P,R,2,8,16] view
            md1, s1, sq11 = squash_a(g, p1[:, :, 0], p1[:, :, 1], 1)
            ab1 = wp_ab(g, p1)                          # sqrt runs under wp/ab
            f1 = squash_b(g, md1, s1, sq11)
            badd(g, ab1, f1, first=True)
            p2 = sig_p(g, last=False)
            md2, s2, sq12 = squash_a(g, p2[:, :, 0], p2[:, :, 1], 2)
            ab2 = wp_ab(g, p2)
            f2 = squash_b(g, md2, s2, sq12)
            badd(g, ab2, f2, first=False)
            p3 = sig_p(g, last=True)                    # [P,R,8,2,16] t-major
            md3, s3, sq13 = squash_a(g, p3[:, :, :, 0], p3[:, :, :, 1], 3)
            f3 = squash_b(g, md3, s3, sq13)
            v3 = grp.tile([P, R, 8, 2, 16], f16, tag="v3")
            for pa in (0, 1):
                nc.vector.tensor_mul(
                    v3[:, :, :, pa], p3[:, :, :, pa],
                    f3[:, :, pa].unsqueeze(3).broadcast_to((P, R, 8, 16)))
            nc.sync.dma_start(
                out=out_d[:, g * R * 256:(g + 1) * R * 256],
                in_=v3[:].rearrange("p a b c d -> p (a b c d)"))
            del st[g]

        for g in range(NG):
            mm_evac(g)
            prefix(g)
            routing(g)

    nc.compile()
    return nc


def _make_in_map(core, shards, w_in, wb_in):
    rb = (core % 4) * 32
    cxy_in = np.zeros((128, 1 + ROWS), np.float32)
    cxy_in[:, 0] = np.arange(128, dtype=np.float32) / 128.0
    cxy_in[:, 1:] = ((rb + np.arange(ROWS, dtype=np.float32)) / 128.0)[None, :]
    return {
        "x_shard": shards[core].astype(np.float16),
        "w_eff": w_in.astype(np.float16),
        "bapp": wb_in.astype(np.float16),
        "cxy": cxy_in,
    }


def kernel(x, W_conv, W_pos, W_app, b_app):
    from concourse.bass_utils import run_bass_kernel_spmd

    if "nc" not in _CACHE:
        _CACHE["nc"] = _build_module()
    nc = _CACHE["nc"]

    w_in, wb_in = _build_weights(W_conv, W_pos, W_app, b_app)
    shards = _shard_x(x)
    in_maps = [_make_in_map(core, shards, w_in, wb_in) for core in range(8)]

    trace = bool(int(os.environ.get("CAPS_TRACE", "0")))
    res = run_bass_kernel_spmd(nc, in_maps, core_ids=list(range(8)), trace=trace)
    _CACHE["last_result"] = res

    out = np.zeros((N, T1, Z, H, W), np.float32)
    for core in range(8):
        n, rb = core // 4, (core % 4) * 32
        o = res.results[core]["out_shard"].astype(np.float32).reshape(
            128, ROWS, 8, 2, 16)
        # [w, r, t, pa, ik] -> [t, pa*16+ik, r, w]
        out[n, :, :, rb:rb + 32, :] = o.transpose(2, 3, 4, 1, 0).reshape(
            8, 32, ROWS, 128)
    return out
